# revision 16
# baseline (speedup 1.0000x reference)
"""Trainium2 Bass kernel for nn_BlendedModel (underwater image formation model).

Math (per pixel, per channel c in [b,g,r] param order paired with x channel c):
  t_c = exp(-sigmoid(alpha_c) * dep)
  back_c = (b_c + (1-b_c)*noise) * (1-t_c);  cb_c = b_c * (1-t_c)
  adaptive gaussian blur: per-pixel kernel, weights u^(i^2+j^2) with
    u = exp(-q), q = 1/(2*(relu(sigma_k+0.001)*dep)^2), normalized by S^2,
    S = 1 + 2*(u + u^4 + u^9 + u^16).
  blur_raw = x + sum_k u^k * C_k;  C_k = sum of shifted pair-sums (i^2+j^2=k)
  blurred = blur_raw * (1/S^2) * t_c
  outputs: (blurred+back, x*t_c + cb, blurred + cb)

Sharding: data-parallel over H (32 rows x 8 cores), partitions = 4 batches x
32 rows, free = 3 channels x (W + 2*HALO) with zero column pads so horizontal
shifts are plain AP offsets. Vertical shifts come from host-staged row-shifted
slabs (pure slicing, same as the fp32 baseline) in bf16.

The rel-err budget is 2e-2, so the whole datapath runs in bf16 (~4e-3 noise)
and blur terms are pruned with a cumulative-contribution budget computed on
the host from the actual inputs (terms are only dropped while the sum of
their true max contributions stays under PRUNE_BUDGET).

Weighted blur terms accumulate on the PE (identity matmuls into PSUM); maps
run on ScalarE; pair-sums/muls are split across VectorE and GpSimd per the
cost model; outputs are stored as bf16 (upcast on host).
"""

import os
import numpy as np
import ml_dtypes

B, C, H, W = 4, 3, 256, 256
NCORES = 8
RPC = H // NCORES          # rows per core
FLAT = C * W               # 768

LAST_EXEC_NS = None

K14 = [1, 2, 4, 5, 8, 9, 10, 13, 16, 17, 18, 20, 25, 32]
CK_PAIRS = {
    1: [(0, 1), (1, 0)], 2: [(1, 1)], 4: [(0, 2), (2, 0)], 5: [(1, 2), (2, 1)],
    8: [(2, 2)], 9: [(0, 3), (3, 0)], 10: [(1, 3), (3, 1)], 13: [(2, 3), (3, 2)],
    16: [(0, 4), (4, 0)], 17: [(1, 4), (4, 1)], 18: [(3, 3)], 20: [(2, 4), (4, 2)],
    25: [(3, 4), (4, 3)], 32: [(4, 4)],
}
# cumulative abs-contribution budget for dropped blur terms (out absmax ~1,
# tolerance 2e-2; bf16 noise eats ~4e-3, leave the rest as margin)
PRUNE_BUDGET = float(os.environ.get("PRUNE_BUDGET", "4.0e-3"))
# S-chain term j^2 is kept while its max relative effect exceeds this
S_EPS = float(os.environ.get("S_EPS", "1.2e-3"))


def _patch_tile_wait_split(tile_mod):
    """This walrus build encodes at most ONE sync-wait per instruction
    (setupSyncWait raises 'Too many sync wait commands'). Split Tile's
    multi-waits onto same-engine NOPs issued immediately before the
    instruction (engine queues are strict FIFO, so semantics match).
    """
    if getattr(tile_mod.TileContext, "_wait_split_patched", False):
        return
    from bass_rust import ScopedClock, SyncInfo

    TC = tile_mod.TileContext
    orig_add = TC._add_instruction

    def _hoist_extra_waits(self, inst):
        si = getattr(inst, "sync_info", None)
        if si is None or not si.on_wait or len(si.on_wait) <= 1:
            return
        waits = list(si.on_wait)
        si.on_wait = waits[-1:]
        eng = self.nc.engines[inst.engine]
        for w in waits[:-1]:
            nop = eng.nop()
            nsi = nop.ins.sync_info
            if nsi is None:
                nop.ins.sync_info = SyncInfo(on_wait=[w], on_update=[])
            else:
                nsi.on_wait = [w]

    def patched_add(self, inst):
        _hoist_extra_waits(self, inst)
        orig_add(self, inst)

    def patched_drain(self, tick_clock, wait_clock):
        drain_inst = self.nc.sync.drain()
        wait_clock.add_sem_waits(
            drain_inst.ins, ScopedClock({None: tick_clock.global_clock})
        )
        si = drain_inst.ins.sync_info
        waits = list(si.on_wait) if si is not None and si.on_wait else []
        if len(waits) > 1:
            si.on_wait = waits[:1]
            for w in waits[1:]:
                nop = self.nc.sync.nop()
                nsi = nop.ins.sync_info
                if nsi is None:
                    nop.ins.sync_info = SyncInfo(on_wait=[w], on_update=[])
                else:
                    nsi.on_wait = [w]
        self.nc.all_engine_barrier()
        popped = self.nc._tile_sem_poison_stack.pop()
        assert popped is self._sem_poison
        self.nc.clear_and_free_semaphores(list(self.sems.allocated().values()))
        self.nc.all_engine_barrier()

    TC._add_instruction = patched_add
    TC._drain_and_barrier = patched_drain
    TC._wait_split_patched = True


def _select_terms(x, dep, c_const):
    """Host-side pruning: drop the smallest-contribution terms while their
    cumulative true max contribution to `blurred` stays under PRUNE_BUDGET.
    Also returns u_max for the adaptive S-chain."""
    d = dep[:, 0].astype(np.float64)
    q = c_const / np.maximum(d * d, 1e-30)
    u = np.exp(-q)
    u_max = float(u.max())
    s_ = 1.0 + 2.0 * (u + u ** 4 + u ** 9 + u ** 16)
    inv_s2 = 1.0 / (s_ * s_)

    def shift(z, dv, dh):
        out = np.zeros_like(z)
        h0, h1 = max(dv, 0), min(H + dv, H)
        w0, w1 = max(dh, 0), min(W + dh, W)
        out[..., h0:h1, w0:w1] = z[..., h0 - dv:h1 - dv, w0 - dh:w1 - dh]
        return out

    xm = x.astype(np.float64)
    contribs = {}
    for k in K14:
        ck = np.zeros_like(xm)
        for (dv, dh) in CK_PAIRS[k]:
            for sv in ({-dv, dv} if dv else {0}):
                for sh in ({-dh, dh} if dh else {0}):
                    ck += shift(xm, sv, sh)
        contribs[k] = float((np.abs(u ** k * inv_s2)[:, None] * np.abs(ck)).max())
    order = sorted(K14, key=lambda k: contribs[k])
    dropped, budget = set(), PRUNE_BUDGET
    for k in order:
        if contribs[k] <= budget:
            dropped.add(k)
            budget -= contribs[k]
        else:
            break
    kept = [k for k in K14 if k not in dropped]
    return kept, u_max


def _build_nc(c_const, a_par, b_par, kept, s_js):
    import concourse.bass as bass
    import concourse.tile as tile
    from concourse import mybir

    _patch_tile_wait_split(tile)
    FP = mybir.dt.float32
    BF = mybir.dt.bfloat16
    ADD = mybir.AluOpType.add
    SUB = mybir.AluOpType.subtract
    MUL = mybir.AluOpType.mult
    Exp = mybir.ActivationFunctionType.Exp
    Copy = mybir.ActivationFunctionType.Copy
    Square = mybir.ActivationFunctionType.Square

    need_v = sorted({d for k in kept for (d, _dp) in CK_PAIRS[k] if d})
    shifts = sorted({0} | {d for d in need_v} | {-d for d in need_v})
    halo = max((dp for k in kept for (_d, dp) in CK_PAIRS[k]), default=1)
    halo = max(halo, 1)
    seg = W + 2 * halo
    free = C * seg

    # base u-maps computed as direct exps on ACT; others chained from them
    act_us = sorted(set(k for k in kept if k in (1, 4, 9, 16)) | {1} |
                    set(j for j in s_js if j not in (2, 5)))
    chain_us = [k for k in sorted(set(kept) | set(s_js)) if k not in act_us]

    dbg = bool(os.environ.get("KDBG"))
    nc = bass.Bass()
    xsh = nc.declare_dram_parameter("xsh", [len(shifts), 128, FLAT], BF, isOutput=False)
    dep = nc.declare_dram_parameter("dep", [128, W], FP, isOutput=False)
    noise = nc.declare_dram_parameter("noise", [128, W], BF, isOutput=False)
    o_out = nc.declare_dram_parameter("out", [128, FLAT], BF, isOutput=True)
    o_co = nc.declare_dram_parameter("clear_out", [128, FLAT], BF, isOutput=True)
    o_cf = nc.declare_dram_parameter("cf", [128, FLAT], BF, isOutput=True)
    eye = nc.declare_dram_parameter("eye", [128, 128], BF, isOutput=False)

    with tile.TileContext(nc) as tc:
        with (
            nc.allow_low_precision("2e-2 output tolerance; bf16 datapath"),
            tc.tile_pool(name="persist", bufs=1) as P,
            tc.tile_pool(name="ps", bufs=1, space="PSUM") as PSp,
        ):
            def ftile(tag, dt=BF):     # padded fused tile (shiftable sources)
                return P.tile([128, free], dt, tag=tag, name=tag)

            def gtile(tag, dt=BF):     # flat fused tile, no pads
                return P.tile([128, FLAT], dt, tag=tag, name=tag)

            def stile(tag, dt=BF):     # single-channel map
                return P.tile([128, W], dt, tag=tag, name=tag)

            def d3(t):                 # [128, 3, W] view of a flat tile
                return t[:].rearrange("p (c s) -> p c s", c=C)

            def dpad(t):               # [128, 3, W] data view of a padded tile
                return t[:].rearrange("p (c s) -> p c s", c=C)[:, :, halo:halo + W]

            def hsh(t, dp):            # [128, 3, W] view shifted by dp columns
                return t[:].rearrange("p (c s) -> p c s", c=C)[:, :, halo + dp:halo + dp + W]

            def bcast(t):              # map broadcast across 3 segments
                return t[:].rearrange("p (o s) -> p o s", o=1).broadcast_to((128, C, W))

            DVE, POOL = nc.vector, nc.gpsimd

            def fresh_padded(tag):
                t = ftile(tag)
                pads = t[:].rearrange("p (c s) -> p c s", c=C)
                nc.gpsimd.memset(pads[:, :, 0:halo], 0.0)
                nc.gpsimd.memset(pads[:, :, seg - halo:seg], 0.0)
                return t

            # ACT-table priming: tiny exp with no deps, overlaps the loads
            dummy = P.tile([128, 1], FP, tag="dummy", name="dummy")
            nc.vector.memset(dummy[:], 0.0)
            nc.scalar.activation(dummy[:], dummy[:], Exp)

            slab = {s: fresh_padded(f"s{s}") for s in shifts}
            sview = {s: xsh[shifts.index(s)].rearrange("p (c s) -> p c s", c=C)
                     for s in shifts}
            # SP queue: dep first (gates the ACT chain), then x slabs
            dpt = stile("dpt", FP)
            nc.sync.dma_start(dpt[:], dep[:])
            pool_shifts = [s for s in shifts if abs(s) >= 2]
            sp_shifts = ([s for s in shifts if abs(s) == 1] + [0])
            for s in sp_shifts:
                nc.sync.dma_start(dpad(slab[s]), sview[s])
            nst = stile("nst", BF)
            nc.sync.dma_start(nst[:], noise[:])
            # Pool queue: far slabs + eye early (no deps; Pool compute later)
            for s in pool_shifts:
                nc.gpsimd.dma_start(dpad(slab[s]), sview[s])
            eyet = P.tile([128, 128], BF, tag="eyet", name="eyet")
            nc.gpsimd.dma_start(eyet[:], eye[:])
            xc = slab[0]

            # ---- engine balancer for TT-class work ---------------------
            # pre-charged with the forced per-engine work: DVE gets recips,
            # omt, co/cf STTs, m_all, blurred(psum), out; Pool gets its queue
            # DMAs, bn, inv/nrm squares, memsets.
            load = {"D": 3.9, "P": 3.1}

            def bal_tt(out_ap, a_ap, b_ap, op, frac=1.0, force=None):
                """emit tensor_tensor on the lighter of DVE/Pool"""
                if force == "D" or (force is None and
                                    load["D"] + 0.46 * frac <=
                                    load["P"] + 0.64 * frac):
                    load["D"] += 0.46 * frac
                    DVE.tensor_tensor(out_ap, a_ap, b_ap, op)
                else:
                    load["P"] += 0.64 * frac
                    POOL.tensor_tensor(out_ap, a_ap, b_ap, op)



            # ---- vertical pair tiles V_d = s[-d] + s[+d] --------------------
            V = {}
            for d in need_v:
                V[d] = fresh_padded(f"V{d}")
                bal_tt(dpad(V[d]), dpad(slab[-d]), dpad(slab[d]), ADD)

            # pre-emit all horizontal pair images in dependency order
            hp = {}

            def get_hpair(d, dp):
                if (d, dp) in hp:
                    return hp[(d, dp)]
                src_t = xc if d == 0 else V[d]
                t = gtile(f"pp{d}_{dp}")
                bal_tt(d3(t), hsh(src_t, -dp), hsh(src_t, dp), ADD)
                hp[(d, dp)] = t
                return t

            for k in kept:
                for (d, dp) in CK_PAIRS[k]:
                    if dp > 0:
                        get_hpair(d, dp)

            # ---- scalar maps -------------------------------------------------
            # inv = 1/dep^2 computed as (1/dep)^2 (no ACT hop on the chain)
            rdep = stile("rdep", FP)
            nc.vector.reciprocal(rdep[:], dpt[:])
            inv = stile("inv", FP)
            nc.gpsimd.tensor_tensor(inv[:], rdep[:], rdep[:], MUL)
            u = {}
            for k in act_us:
                u[k] = stile(f"u{k}")
                nc.scalar.activation(u[k][:], inv[:], Exp, scale=float(-k * c_const))
            for k in chain_us:
                have = sorted(u.keys())
                found = None
                for a in have:
                    for bb in have:
                        if a + bb == k:
                            found = (a, bb)
                            break
                    if found:
                        break
                u[k] = stile(f"u{k}")
                if found:
                    bal_tt(u[k][:], u[found[0]][:], u[found[1]][:], MUL, frac=0.33)
                else:
                    nc.scalar.activation(u[k][:], inv[:], Exp,
                                         scale=float(-k * c_const))
            # t_all = exp(-a_c * dep) per channel
            t_all = gtile("t_all")
            for ci in range(C):
                nc.scalar.activation(d3(t_all)[:, ci, :], dpt[:], Exp,
                                     scale=float(-a_par[ci]))
            # S-chain off ACT: S = 1 + 2*sum(u^j); nrm = (1/S)^2
            sj = [u[j] for j in s_js]
            if len(sj) == 1:
                s_acc = sj[0]
            else:
                s_acc = stile("s_acc")
                nc.gpsimd.tensor_tensor(s_acc[:], sj[0][:], sj[1][:], ADD)
                for extra in sj[2:]:
                    nc.gpsimd.tensor_tensor(s_acc[:], s_acc[:], extra[:], ADD)
            svt = stile("svt", FP)
            nc.vector.tensor_scalar(svt[:], s_acc[:], 2.0, 1.0, MUL, ADD)
            rS = stile("rS", FP)
            nc.vector.reciprocal(rS[:], svt[:])
            nrm = stile("nrm")
            nc.gpsimd.tensor_tensor(nrm[:], rS[:], rS[:], MUL)

            # ---- blur terms --------------------------------------------------
            acc_ps = PSp.tile([128, FLAT], FP, tag="acc_ps", name="acc_ps")
            CHUNKS = [(0, 512), (512, 256)]

            def pe_add(dst_ps, src_flat, start, stop):
                for (o, n) in CHUNKS:
                    nc.tensor.matmul(
                        dst_ps[:, o:o + n], eyet[:], src_flat[:, o:o + n],
                        start=start, stop=stop, skip_group_check=True,
                    )

            npe_total = len(kept) + 1
            npe = [0]

            def accum(src_flat):
                npe[0] += 1
                pe_add(acc_ps, src_flat, start=(npe[0] == 1),
                       stop=(npe[0] == npe_total))

            def accum_3d(src_3d):
                # start=True (first_mm) clears the whole PSUM *bank*: set it
                # only on the first matmul touching each 512-col bank
                # (ci=0 -> bank 0, ci=2 -> bank 1; ci=1 shares bank 0).
                npe[0] += 1
                st, sp = (npe[0] == 1), (npe[0] == npe_total)
                for ci in range(C):
                    nc.tensor.matmul(
                        acc_ps[:, ci * W:(ci + 1) * W], eyet[:], src_3d[:, ci, :],
                        start=(st and ci != 1), stop=sp, skip_group_check=True,
                    )

            # x itself is the k=0 term
            accum_3d(dpad(xc))

            # B-routes: C_k built on the PE via shifted-AP identity matmuls
            # (mul then reads PSUM on DVE at 1x). Applied to k=2-style
            # (single pair (d,dp), d in V) and k=4-style ((0,dp)+(d,0)).
            def b2_style(k):
                return False

            def b4_style(k):
                return False

            def mm_shift(dst_ps, src, dp, start, stop):
                for ci in range(C):
                    nc.tensor.matmul(
                        dst_ps[:, ci * W:(ci + 1) * W], eyet[:],
                        hsh(src, dp)[:, ci, :] if dp else dpad(src)[:, ci, :],
                        start=start, stop=stop, skip_group_check=True,
                    )

            for k in kept:
                pairs = CK_PAIRS[k]
                mk = gtile(f"mk{k}")
                parts = []
                for (d, dp) in pairs:
                    parts.append(V[d] if dp == 0 else get_hpair(d, dp))
                if len(parts) == 1:
                    ck = parts[0]
                else:
                    ck = gtile(f"ck{k}")
                    aps = [d3(p) if p.shape[1] == FLAT else dpad(p)
                           for p in parts]
                    bal_tt(d3(ck), aps[0], aps[1], ADD)
                src_ap = d3(ck) if ck.shape[1] == FLAT else dpad(ck)
                bal_tt(d3(mk), src_ap, bcast(u[k]), MUL)
                accum(mk[:])

            # ---- epilogue ----------------------------------------------------
            # bn_c = b_c + (1-b_c)*noise ; omt = 1 - t ; back = bn*omt
            bn = gtile("bn")
            for ci in range(C):
                nc.gpsimd.tensor_scalar(d3(bn)[:, ci, :], nst[:],
                                        float(1.0 - b_par[ci]), float(b_par[ci]),
                                        MUL, ADD)
            omt = gtile("omt")
            nc.vector.tensor_scalar(omt[:], t_all[:], -1.0, 1.0, MUL, ADD)
            back = gtile("back")
            bal_tt(back[:], bn[:], omt[:], MUL)

            # clear_out = x*t + b_c*omt (early, no blur dependency)
            co_t = gtile("co_t")
            for ci in range(C):
                nc.vector.scalar_tensor_tensor(
                    d3(co_t)[:, ci, :], dpad(xc)[:, ci, :], float(b_par[ci]),
                    d3(t_all)[:, ci, :], SUB, MUL)
            for ci in range(C):
                nc.scalar.activation(d3(co_t)[:, ci, :], d3(co_t)[:, ci, :],
                                     Copy, bias=float(b_par[ci]))
            nc.sync.dma_start(o_co[:], co_t[:])

            # m_all = t / S^2 ; blurred = acc * m_all (PSUM read: DVE only)
            m_all = gtile("m_all")
            bal_tt(d3(m_all), d3(t_all), bcast(nrm), MUL, force="D")
            blurred = gtile("blurred")
            nc.vector.tensor_tensor(blurred[:], m_all[:], acc_ps[:], MUL)

            # out = blurred + back (store on SP)
            out_t = gtile("out_t")
            bal_tt(out_t[:], blurred[:], back[:], ADD, force="D")
            nc.sync.dma_start(o_out[:], out_t[:])

            # cf_c = blurred_c + b_c*omt_c via per-channel STT (kills cb)
            cf_t = gtile("cf_t")
            for ci in range(C):
                nc.vector.scalar_tensor_tensor(
                    d3(cf_t)[:, ci, :], d3(omt)[:, ci, :], float(b_par[ci]),
                    d3(blurred)[:, ci, :], MUL, ADD)
            nc.gpsimd.dma_start(o_cf[:], cf_t[:])

            if dbg:
                dbg_specs = {"d_u1": u[kept[0]], "d_nrm": nrm, "d_mall": m_all,
                             "d_v1": V[need_v[0]] if need_v else m_all,
                             "d_mk": mk, "d_blur": blurred}
                acc_sb = gtile("acc_sb", FP)
                nc.scalar.activation(acc_sb[:], acc_ps[:], Copy)
                dram_accs = nc.declare_dram_parameter("d_acc", [128, FLAT], FP,
                                                      isOutput=True)
                nc.sync.dma_start(dram_accs[:], acc_sb[:])
                for nm, t in dbg_specs.items():
                    pr = nc.declare_dram_parameter(nm, list(t.shape), t.dtype,
                                                   isOutput=True)
                    nc.sync.dma_start(pr[:], t[:])

    return nc


def prepare(x, dep, noise, sigma_k, alpha_r, b_r, alpha_g, b_g, alpha_b, b_b):
    """Build the Bass program + per-core input maps for the given inputs."""
    x = np.ascontiguousarray(x, np.float32)
    dep = np.ascontiguousarray(dep, np.float32)
    noise = np.ascontiguousarray(noise, np.float32)

    sig = lambda v: 1.0 / (1.0 + np.exp(-np.float64(v)))
    # output channel order [b, g, r] pairs with x channels [0, 1, 2]
    a_par = [float(sig(alpha_b[0])), float(sig(alpha_g[0])), float(sig(alpha_r[0]))]
    b_par = [float(sig(b_b[0])), float(sig(b_g[0])), float(sig(b_r[0]))]
    kk = max(float(np.float32(sigma_k[0]) + np.float32(0.001)), 0.0)
    c_const = float(1.0 / (2.0 * np.float64(kk) * np.float64(kk)))

    kept, u_max = _select_terms(x, dep, c_const)
    if not kept:
        kept = [1]
    # adaptive S-chain: keep u^(j^2) while its effect on S is above S_EPS
    s_js = [j * j for j in (1, 2, 3, 4) if 2.0 * (u_max ** (j * j)) > S_EPS]
    if not s_js:
        s_js = [1]
    nc = _build_nc(c_const, a_par, b_par, kept, s_js)

    need_v = sorted({d for k in kept for (d, _dp) in CK_PAIRS[k] if d})
    shifts = sorted({0} | {d for d in need_v} | {-d for d in need_v})

    HALO_V = max(need_v, default=0)
    xp = np.pad(x, ((0, 0), (0, 0), (HALO_V, HALO_V), (0, 0)))
    xp = xp.astype(ml_dtypes.bfloat16)
    in_maps = []
    for i in range(NCORES):
        r0 = i * RPC
        slabs = np.empty((len(shifts), 128, FLAT), ml_dtypes.bfloat16)
        for si, s in enumerate(shifts):
            blk = xp[:, :, r0 + HALO_V + s: r0 + HALO_V + s + RPC]   # (B,C,RPC,W)
            slabs[si] = blk.transpose(0, 2, 1, 3).reshape(128, FLAT)
        in_maps.append({
            "xsh": slabs,
            "dep": np.ascontiguousarray(dep[:, 0, r0:r0 + RPC]).reshape(128, W),
            "noise": np.ascontiguousarray(noise[:, 0, r0:r0 + RPC]).reshape(
                128, W).astype(ml_dtypes.bfloat16),
            "eye": np.eye(128, dtype=ml_dtypes.bfloat16),
        })
    return nc, in_maps


def kernel(x, dep, noise, sigma_k, alpha_r, b_r, alpha_g, b_g, alpha_b, b_b):
    from concourse.bass_utils import run_bass_kernel_spmd

    nc, in_maps = prepare(x, dep, noise, sigma_k, alpha_r, b_r, alpha_g, b_g,
                          alpha_b, b_b)
    res = run_bass_kernel_spmd(nc, in_maps, list(range(NCORES)))
    global LAST_EXEC_NS
    LAST_EXEC_NS = getattr(res, "exec_time_ns", None)

    def assemble(name):
        full = np.empty((B, C, H, W), np.float32)
        for i in range(NCORES):
            blk = np.asarray(res.results[i][name], np.float32)
            blk = blk.reshape(B, RPC, C, W).transpose(0, 2, 1, 3)
            full[:, :, i * RPC:(i + 1) * RPC] = blk
        return full

    return assemble("out"), assemble("clear_out"), assemble("cf")


# revision 27
# speedup vs baseline: 1.1186x; 1.1186x over previous
"""Trainium2 Bass kernel for nn_BlendedModel (underwater image formation model).

Math (per pixel, per channel c in [b,g,r] param order paired with x channel c):
  t_c = exp(-sigmoid(alpha_c) * dep)
  back_c = (b_c + (1-b_c)*noise) * (1-t_c);  cb_c = b_c * (1-t_c)
  adaptive gaussian blur: per-pixel kernel, weights u^(i^2+j^2) with
    u = exp(-q), q = 1/(2*(relu(sigma_k+0.001)*dep)^2), normalized by S^2,
    S = 1 + 2*(u + u^4 + u^9 + u^16).
  blur_raw = x + sum_k u^k * C_k;  C_k = sum of shifted pair-sums (i^2+j^2=k)
  blurred = blur_raw * (1/S^2) * t_c
  outputs: (blurred+back, x*t_c + cb, blurred + cb)

Sharding: data-parallel over H (32 rows x 8 cores), partitions = 4 batches x
32 rows, free = 3 channels x (W + 2*HALO) with zero column pads so horizontal
shifts are plain AP offsets. Vertical shifts come from host-staged row-shifted
slabs (pure slicing, same as the fp32 baseline) in bf16.

The rel-err budget is 2e-2, so the whole datapath runs in bf16 (~4e-3 noise)
and blur terms are pruned with a cumulative-contribution budget computed on
the host from the actual inputs (terms are only dropped while the sum of
their true max contributions stays under PRUNE_BUDGET).

Weighted blur terms accumulate on the PE (identity matmuls into PSUM); maps
run on ScalarE; pair-sums/muls are split across VectorE and GpSimd per the
cost model; outputs are stored as bf16 (upcast on host).
"""

import os
import numpy as np
import ml_dtypes

B, C, H, W = 4, 3, 256, 256
NCORES = 8
RPC = H // NCORES          # rows per core
FLAT = C * W               # 768

LAST_EXEC_NS = None

K14 = [1, 2, 4, 5, 8, 9, 10, 13, 16, 17, 18, 20, 25, 32]
CK_PAIRS = {
    1: [(0, 1), (1, 0)], 2: [(1, 1)], 4: [(0, 2), (2, 0)], 5: [(1, 2), (2, 1)],
    8: [(2, 2)], 9: [(0, 3), (3, 0)], 10: [(1, 3), (3, 1)], 13: [(2, 3), (3, 2)],
    16: [(0, 4), (4, 0)], 17: [(1, 4), (4, 1)], 18: [(3, 3)], 20: [(2, 4), (4, 2)],
    25: [(3, 4), (4, 3)], 32: [(4, 4)],
}
# cumulative abs-contribution budget for dropped blur terms (out absmax ~1,
# tolerance 2e-2; bf16 noise eats ~4e-3, leave the rest as margin)
PRUNE_BUDGET = float(os.environ.get("PRUNE_BUDGET", "4.0e-3"))
# S-chain term j^2 is kept while its max relative effect exceeds this
S_EPS = float(os.environ.get("S_EPS", "1.2e-3"))


def _patch_tile_wait_split(tile_mod):
    """This walrus build encodes at most ONE sync-wait per instruction
    (setupSyncWait raises 'Too many sync wait commands'). Split Tile's
    multi-waits onto same-engine NOPs issued immediately before the
    instruction (engine queues are strict FIFO, so semantics match).
    """
    if getattr(tile_mod.TileContext, "_wait_split_patched", False):
        return
    from bass_rust import ScopedClock, SyncInfo

    TC = tile_mod.TileContext
    orig_add = TC._add_instruction

    def _hoist_extra_waits(self, inst):
        si = getattr(inst, "sync_info", None)
        if si is None or not si.on_wait or len(si.on_wait) <= 1:
            return
        waits = list(si.on_wait)
        si.on_wait = waits[-1:]
        eng = self.nc.engines[inst.engine]
        for w in waits[:-1]:
            nop = eng.nop()
            nsi = nop.ins.sync_info
            if nsi is None:
                nop.ins.sync_info = SyncInfo(on_wait=[w], on_update=[])
            else:
                nsi.on_wait = [w]

    def patched_add(self, inst):
        _hoist_extra_waits(self, inst)
        orig_add(self, inst)

    def patched_drain(self, tick_clock, wait_clock):
        drain_inst = self.nc.sync.drain()
        wait_clock.add_sem_waits(
            drain_inst.ins, ScopedClock({None: tick_clock.global_clock})
        )
        si = drain_inst.ins.sync_info
        waits = list(si.on_wait) if si is not None and si.on_wait else []
        if len(waits) > 1:
            si.on_wait = waits[:1]
            for w in waits[1:]:
                nop = self.nc.sync.nop()
                nsi = nop.ins.sync_info
                if nsi is None:
                    nop.ins.sync_info = SyncInfo(on_wait=[w], on_update=[])
                else:
                    nsi.on_wait = [w]
        self.nc.all_engine_barrier()
        popped = self.nc._tile_sem_poison_stack.pop()
        assert popped is self._sem_poison
        self.nc.clear_and_free_semaphores(list(self.sems.allocated().values()))
        self.nc.all_engine_barrier()

    TC._add_instruction = patched_add
    TC._drain_and_barrier = patched_drain
    TC._wait_split_patched = True


def _select_terms(x, dep, c_const):
    """Host-side pruning: drop the smallest-contribution terms while their
    cumulative true max contribution to `blurred` stays under PRUNE_BUDGET.
    Also returns u_max for the adaptive S-chain."""
    d = dep[:, 0].astype(np.float64)
    q = c_const / np.maximum(d * d, 1e-30)
    u = np.exp(-q)
    u_max = float(u.max())
    s_ = 1.0 + 2.0 * (u + u ** 4 + u ** 9 + u ** 16)
    inv_s2 = 1.0 / (s_ * s_)

    def shift(z, dv, dh):
        out = np.zeros_like(z)
        h0, h1 = max(dv, 0), min(H + dv, H)
        w0, w1 = max(dh, 0), min(W + dh, W)
        out[..., h0:h1, w0:w1] = z[..., h0 - dv:h1 - dv, w0 - dh:w1 - dh]
        return out

    xm = x.astype(np.float64)
    contribs = {}
    for k in K14:
        ck = np.zeros_like(xm)
        for (dv, dh) in CK_PAIRS[k]:
            for sv in ({-dv, dv} if dv else {0}):
                for sh in ({-dh, dh} if dh else {0}):
                    ck += shift(xm, sv, sh)
        contribs[k] = float((np.abs(u ** k * inv_s2)[:, None] * np.abs(ck)).max())
    order = sorted(K14, key=lambda k: contribs[k])
    dropped, budget = set(), PRUNE_BUDGET
    for k in order:
        if contribs[k] <= budget:
            dropped.add(k)
            budget -= contribs[k]
        else:
            break
    kept = [k for k in K14 if k not in dropped]
    return kept, u_max


def _build_nc(c_const, a_par, b_par, kept, s_js):
    import concourse.bass as bass
    import concourse.tile as tile
    from concourse import mybir

    _patch_tile_wait_split(tile)
    FP = mybir.dt.float32
    BF = mybir.dt.bfloat16
    ADD = mybir.AluOpType.add
    SUB = mybir.AluOpType.subtract
    MUL = mybir.AluOpType.mult
    Exp = mybir.ActivationFunctionType.Exp
    Copy = mybir.ActivationFunctionType.Copy
    Square = mybir.ActivationFunctionType.Square

    need_v = sorted({d for k in kept for (d, _dp) in CK_PAIRS[k] if d})
    shifts = sorted({0} | {d for d in need_v} | {-d for d in need_v})
    halo = max((dp for k in kept for (_d, dp) in CK_PAIRS[k]), default=1)
    halo = max(halo, 1)
    seg = W + 2 * halo
    free = C * seg

    # base u-maps computed as direct exps on ACT; others chained from them
    act_us = sorted(set(k for k in kept if k in (1, 4, 9, 16)) | {1} |
                    set(j for j in s_js if j not in (2, 5)))
    chain_us = [k for k in sorted(set(kept) | set(s_js)) if k not in act_us]

    dbg = bool(os.environ.get("KDBG"))
    nc = bass.Bass()
    xsh = nc.declare_dram_parameter("xsh", [len(shifts), 128, FLAT], BF, isOutput=False)
    dep = nc.declare_dram_parameter("dep", [128, W], FP, isOutput=False)
    noise = nc.declare_dram_parameter("noise", [128, W], BF, isOutput=False)
    o_out = nc.declare_dram_parameter("out", [128, FLAT], BF, isOutput=True)
    o_co = nc.declare_dram_parameter("clear_out", [128, FLAT], BF, isOutput=True)
    o_cf = nc.declare_dram_parameter("cf", [128, FLAT], BF, isOutput=True)
    eye = nc.declare_dram_parameter("eye", [128, 128], BF, isOutput=False)

    with tile.TileContext(nc) as tc:
        with (
            nc.allow_low_precision("2e-2 output tolerance; bf16 datapath"),
            tc.tile_pool(name="persist", bufs=1) as P,
            tc.tile_pool(name="ps", bufs=1, space="PSUM") as PSp,
        ):
            def ftile(tag, dt=BF):     # padded fused tile (shiftable sources)
                return P.tile([128, free], dt, tag=tag, name=tag)

            def gtile(tag, dt=BF):     # flat fused tile, no pads
                return P.tile([128, FLAT], dt, tag=tag, name=tag)

            def stile(tag, dt=BF):     # single-channel map
                return P.tile([128, W], dt, tag=tag, name=tag)

            def d3(t):                 # [128, 3, W] view of a flat tile
                return t[:].rearrange("p (c s) -> p c s", c=C)

            def dpad(t):               # [128, 3, W] data view of a padded tile
                return t[:].rearrange("p (c s) -> p c s", c=C)[:, :, halo:halo + W]

            def hsh(t, dp):            # [128, 3, W] view shifted by dp columns
                return t[:].rearrange("p (c s) -> p c s", c=C)[:, :, halo + dp:halo + dp + W]

            def bcast(t):              # map broadcast across 3 segments
                return t[:].rearrange("p (o s) -> p o s", o=1).broadcast_to((128, C, W))

            DVE, POOL = nc.vector, nc.gpsimd

            def fresh_padded(tag):
                t = ftile(tag)
                pads = t[:].rearrange("p (c s) -> p c s", c=C)
                nc.gpsimd.memset(pads[:, :, 0:halo], 0.0)
                nc.gpsimd.memset(pads[:, :, seg - halo:seg], 0.0)
                return t

            # ACT-table priming: tiny exp with no deps, overlaps the loads
            dummy = P.tile([128, 1], FP, tag="dummy", name="dummy")
            nc.vector.memset(dummy[:], 0.0)
            nc.scalar.activation(dummy[:], dummy[:], Exp)

            slab = {s: fresh_padded(f"s{s}") for s in shifts}
            sview = {s: xsh[shifts.index(s)].rearrange("p (c s) -> p c s", c=C)
                     for s in shifts}
            # SP queue: dep first (gates the ACT chain), then x slabs
            dpt = stile("dpt", FP)
            nc.sync.dma_start(dpt[:], dep[:])
            pool_shifts = [s for s in shifts if abs(s) >= 2]
            sp_shifts = (sorted([s for s in shifts if abs(s) == 1]) + [0])
            for s in sp_shifts:
                nc.sync.dma_start(dpad(slab[s]), sview[s])
            nst = stile("nst", BF)
            nc.sync.dma_start(nst[:], noise[:])
            # Pool queue: far slabs + eye early (no deps; Pool compute later)
            for s in pool_shifts:
                nc.gpsimd.dma_start(dpad(slab[s]), sview[s])
            eyet = P.tile([128, 128], BF, tag="eyet", name="eyet")
            nc.gpsimd.dma_start(eyet[:], eye[:])
            xc = slab[0]

            # ---- engine balancer for TT-class work ---------------------
            # pre-charged with the forced per-engine work: DVE gets recips,
            # omt, co/cf STTs, m_all, blurred(psum), out; Pool gets its queue
            # DMAs, bn, inv/nrm squares, memsets.
            load = {"D": 3.6, "P": 3.0}

            def bal_tt(out_ap, a_ap, b_ap, op, frac=1.0, force=None):
                """emit tensor_tensor on the lighter of DVE/Pool"""
                if force == "D" or (force is None and
                                    load["D"] + 0.46 * frac <=
                                    load["P"] + 0.64 * frac):
                    load["D"] += 0.46 * frac
                    DVE.tensor_tensor(out_ap, a_ap, b_ap, op)
                else:
                    load["P"] += 0.64 * frac
                    POOL.tensor_tensor(out_ap, a_ap, b_ap, op)



            # ---- scalar maps -------------------------------------------------
            # inv = 1/dep^2 computed as (1/dep)^2 (no ACT hop on the chain)
            rdep = stile("rdep", FP)
            nc.vector.reciprocal(rdep[:], dpt[:])
            inv = stile("inv", FP)
            nc.gpsimd.tensor_tensor(inv[:], rdep[:], rdep[:], MUL)
            u = {}
            for k in act_us:
                u[k] = stile(f"u{k}")
                nc.scalar.activation(u[k][:], inv[:], Exp, scale=float(-k * c_const))
            for k in chain_us:
                have = sorted(u.keys())
                found = None
                for a in have:
                    for bb in have:
                        if a + bb == k:
                            found = (a, bb)
                            break
                    if found:
                        break
                u[k] = stile(f"u{k}")
                if found:
                    bal_tt(u[k][:], u[found[0]][:], u[found[1]][:], MUL, frac=0.33)
                else:
                    nc.scalar.activation(u[k][:], inv[:], Exp,
                                         scale=float(-k * c_const))
            # t_all = exp(-a_c * dep) per channel
            t_all = gtile("t_all")
            for ci in range(C):
                nc.scalar.activation(d3(t_all)[:, ci, :], dpt[:], Exp,
                                     scale=float(-a_par[ci]))
            # S-chain off ACT: S = 1 + 2*sum(u^j); nrm = (1/S)^2
            sj = [u[j] for j in s_js]
            if len(sj) == 1:
                s_acc = sj[0]
            else:
                s_acc = stile("s_acc")
                nc.gpsimd.tensor_tensor(s_acc[:], sj[0][:], sj[1][:], ADD)
                for extra in sj[2:]:
                    nc.gpsimd.tensor_tensor(s_acc[:], s_acc[:], extra[:], ADD)
            svt = stile("svt", FP)
            nc.scalar.activation(svt[:], s_acc[:], Copy, bias=1.0, scale=2.0)
            rS = stile("rS", FP)
            nc.vector.reciprocal(rS[:], svt[:])
            nrm = stile("nrm")
            nc.scalar.activation(nrm[:], rS[:], Square)

            # ---- vertical pair tiles V_d = s[-d] + s[+d] --------------------
            V = {}
            for d in need_v:
                V[d] = fresh_padded(f"V{d}")
                bal_tt(dpad(V[d]), dpad(slab[-d]), dpad(slab[d]), ADD)

            # pre-emit all horizontal pair images in dependency order
            hp = {}

            def get_hpair(d, dp):
                if (d, dp) in hp:
                    return hp[(d, dp)]
                src_t = xc if d == 0 else V[d]
                t = gtile(f"pp{d}_{dp}")
                bal_tt(d3(t), hsh(src_t, -dp), hsh(src_t, dp), ADD)
                hp[(d, dp)] = t
                return t

            for k in kept:
                for (d, dp) in CK_PAIRS[k]:
                    if dp > 0:
                        get_hpair(d, dp)

            # ---- blur terms --------------------------------------------------
            acc_ps = PSp.tile([128, FLAT], FP, tag="acc_ps", name="acc_ps")
            CHUNKS = [(0, 512), (512, 256)]

            def pe_add(dst_ps, src_flat, start, stop):
                for (o, n) in CHUNKS:
                    nc.tensor.matmul(
                        dst_ps[:, o:o + n], eyet[:], src_flat[:, o:o + n],
                        start=start, stop=stop, skip_group_check=True,
                    )

            npe_total = len(kept) + 1
            npe = [0]

            def accum(src_flat):
                npe[0] += 1
                pe_add(acc_ps, src_flat, start=(npe[0] == 1),
                       stop=(npe[0] == npe_total))

            def accum_3d(src_3d):
                # start=True (first_mm) clears the whole PSUM *bank*: set it
                # only on the first matmul touching each 512-col bank
                # (ci=0 -> bank 0, ci=2 -> bank 1; ci=1 shares bank 0).
                npe[0] += 1
                st, sp = (npe[0] == 1), (npe[0] == npe_total)
                for ci in range(C):
                    nc.tensor.matmul(
                        acc_ps[:, ci * W:(ci + 1) * W], eyet[:], src_3d[:, ci, :],
                        start=(st and ci != 1), stop=sp, skip_group_check=True,
                    )

            # x itself is the k=0 term
            accum_3d(dpad(xc))

            # B-routes: C_k built on the PE via shifted-AP identity matmuls
            # (mul then reads PSUM on DVE at 1x). Applied to k=2-style
            # (single pair (d,dp), d in V) and k=4-style ((0,dp)+(d,0)).
            def b2_style(k):
                return False

            def b4_style(k):
                # two-pair k with both pairs shifted in V and columns
                # (e.g. k=5: (1,2)+(2,1)): all four shift-sums on the PE.
                p = CK_PAIRS[k]
                return (len(p) == 2 and all(d in V and dp > 0 for d, dp in p)
                        and os.environ.get("NO_B5") is None)

            def mm_shift(dst_ps, src, dp, start, stop):
                # start=True (first_mm) clears the whole PSUM bank; only the
                # first matmul touching each bank may set it (ci 0 -> bank 0,
                # ci 2 -> bank 1; ci 1 shares bank 0).
                for ci in range(C):
                    nc.tensor.matmul(
                        dst_ps[:, ci * W:(ci + 1) * W], eyet[:],
                        hsh(src, dp)[:, ci, :] if dp else dpad(src)[:, ci, :],
                        start=(start and ci != 1), stop=stop,
                        skip_group_check=True,
                    )

            # B-route shift-matmuls are emitted FIRST so the PE FIFO does
            # not block them behind mk accumulations.
            ckps = {}
            for k in kept:
                if b4_style(k):
                    ck_ps = PSp.tile([128, FLAT], FP, tag=f"ck{k}_ps",
                                     name=f"ck{k}_ps")
                    (da, dpa), (db, dpb) = CK_PAIRS[k]
                    mm_shift(ck_ps, V[da], -dpa, start=True, stop=False)
                    mm_shift(ck_ps, V[da], dpa, start=False, stop=False)
                    mm_shift(ck_ps, V[db], -dpb, start=False, stop=False)
                    mm_shift(ck_ps, V[db], dpb, start=False, stop=True)
                    ckps[k] = ck_ps

            for ki, k in enumerate(kept):
                pairs = CK_PAIRS[k]
                mk = gtile(f"mk{k}")
                if k in ckps:
                    nc.vector.tensor_tensor(d3(mk), d3(ckps[k]), bcast(u[k]),
                                            MUL)
                    load["D"] += 0.925
                    accum(mk[:])
                    continue
                parts = []
                for (d, dp) in pairs:
                    parts.append(V[d] if dp == 0 else get_hpair(d, dp))
                if len(parts) == 1:
                    ck = parts[0]
                else:
                    ck = gtile(f"ck{k}")
                    aps = [d3(p) if p.shape[1] == FLAT else dpad(p)
                           for p in parts]
                    bal_tt(d3(ck), aps[0], aps[1], ADD)
                src_ap = d3(ck) if ck.shape[1] == FLAT else dpad(ck)
                bal_tt(d3(mk), src_ap, bcast(u[k]), MUL)
                accum(mk[:])

            # ---- epilogue ----------------------------------------------------
            # bn_c = b_c + (1-b_c)*noise ; omt = 1 - t ; back = bn*omt
            bn = gtile("bn")
            for ci in range(C):
                nc.gpsimd.tensor_scalar(d3(bn)[:, ci, :], nst[:],
                                        float(1.0 - b_par[ci]), float(b_par[ci]),
                                        MUL, ADD)
            omt = gtile("omt")
            for ci in range(C):
                nc.scalar.activation(d3(omt)[:, ci, :], d3(t_all)[:, ci, :],
                                     Copy, bias=1.0, scale=-1.0)
            back = gtile("back")
            bal_tt(back[:], bn[:], omt[:], MUL)

            # cb_c = b_c*(1 - t_c) via ACT copies (idle mid-kernel)
            cb = gtile("cb")
            for ci in range(C):
                nc.scalar.activation(d3(cb)[:, ci, :], d3(t_all)[:, ci, :],
                                     Copy, bias=float(b_par[ci]),
                                     scale=float(-b_par[ci]))
            # clear_out = x*t + cb (cb shared with cf)
            clear = gtile("clear")
            nc.vector.tensor_tensor(d3(clear), dpad(xc), d3(t_all), MUL)
            load["D"] += 0.46
            co_t = gtile("co_t")
            nc.gpsimd.tensor_tensor(co_t[:], clear[:], cb[:], ADD)
            load["P"] += 0.64
            nc.sync.dma_start(o_co[:], co_t[:])

            HL = [(0, FLAT // 2), (FLAT // 2, FLAT // 2)]
            # m_all right before blurred in the DVE FIFO (overlaps the last
            # PE accumulation)
            m_all = gtile("m_all")
            nc.vector.tensor_tensor(d3(m_all), d3(t_all), bcast(nrm), MUL)
            load["D"] += 0.46
            blurred = gtile("blurred")
            out_t = gtile("out_t")
            cf_t = gtile("cf_t")
            # pipelined halves: blurred half -> out/cf adds + stores overlap
            # with the other half
            for (o, n) in HL:
                nc.vector.tensor_tensor(blurred[:, o:o + n], m_all[:, o:o + n],
                                        acc_ps[:, o:o + n], MUL)
                nc.vector.tensor_tensor(out_t[:, o:o + n], blurred[:, o:o + n],
                                        back[:, o:o + n], ADD)
                nc.sync.dma_start(o_out[:, o:o + n], out_t[:, o:o + n])
                nc.gpsimd.tensor_tensor(cf_t[:, o:o + n], blurred[:, o:o + n],
                                        cb[:, o:o + n], ADD)
                nc.scalar.dma_start(o_cf[:, o:o + n], cf_t[:, o:o + n])
            load["D"] += 0.52
            load["P"] += 0.72



            if dbg:
                dbg_specs = {"d_u1": u[kept[0]], "d_nrm": nrm, "d_mall": m_all,
                             "d_v1": V[need_v[0]] if need_v else m_all,
                             "d_mk": mk, "d_blur": blurred}
                acc_sb = gtile("acc_sb", FP)
                nc.scalar.activation(acc_sb[:], acc_ps[:], Copy)
                dram_accs = nc.declare_dram_parameter("d_acc", [128, FLAT], FP,
                                                      isOutput=True)
                nc.sync.dma_start(dram_accs[:], acc_sb[:])
                for nm, t in dbg_specs.items():
                    pr = nc.declare_dram_parameter(nm, list(t.shape), t.dtype,
                                                   isOutput=True)
                    nc.sync.dma_start(pr[:], t[:])

    return nc


def prepare(x, dep, noise, sigma_k, alpha_r, b_r, alpha_g, b_g, alpha_b, b_b):
    """Build the Bass program + per-core input maps for the given inputs."""
    x = np.ascontiguousarray(x, np.float32)
    dep = np.ascontiguousarray(dep, np.float32)
    noise = np.ascontiguousarray(noise, np.float32)

    sig = lambda v: 1.0 / (1.0 + np.exp(-np.float64(v)))
    # output channel order [b, g, r] pairs with x channels [0, 1, 2]
    a_par = [float(sig(alpha_b[0])), float(sig(alpha_g[0])), float(sig(alpha_r[0]))]
    b_par = [float(sig(b_b[0])), float(sig(b_g[0])), float(sig(b_r[0]))]
    kk = max(float(np.float32(sigma_k[0]) + np.float32(0.001)), 0.0)
    c_const = float(1.0 / (2.0 * np.float64(kk) * np.float64(kk)))

    kept, u_max = _select_terms(x, dep, c_const)
    if not kept:
        kept = [1]
    # adaptive S-chain: keep u^(j^2) while its effect on S is above S_EPS
    s_js = [j * j for j in (1, 2, 3, 4) if 2.0 * (u_max ** (j * j)) > S_EPS]
    if not s_js:
        s_js = [1]
    nc = _build_nc(c_const, a_par, b_par, kept, s_js)

    need_v = sorted({d for k in kept for (d, _dp) in CK_PAIRS[k] if d})
    shifts = sorted({0} | {d for d in need_v} | {-d for d in need_v})

    HALO_V = max(need_v, default=0)
    xp = np.pad(x, ((0, 0), (0, 0), (HALO_V, HALO_V), (0, 0)))
    xp = xp.astype(ml_dtypes.bfloat16)
    in_maps = []
    for i in range(NCORES):
        r0 = i * RPC
        slabs = np.empty((len(shifts), 128, FLAT), ml_dtypes.bfloat16)
        for si, s in enumerate(shifts):
            blk = xp[:, :, r0 + HALO_V + s: r0 + HALO_V + s + RPC]   # (B,C,RPC,W)
            slabs[si] = blk.transpose(0, 2, 1, 3).reshape(128, FLAT)
        in_maps.append({
            "xsh": slabs,
            "dep": np.ascontiguousarray(dep[:, 0, r0:r0 + RPC]).reshape(128, W),
            "noise": np.ascontiguousarray(noise[:, 0, r0:r0 + RPC]).reshape(
                128, W).astype(ml_dtypes.bfloat16),
            "eye": np.eye(128, dtype=ml_dtypes.bfloat16),
        })
    return nc, in_maps


def kernel(x, dep, noise, sigma_k, alpha_r, b_r, alpha_g, b_g, alpha_b, b_b):
    from concourse.bass_utils import run_bass_kernel_spmd

    nc, in_maps = prepare(x, dep, noise, sigma_k, alpha_r, b_r, alpha_g, b_g,
                          alpha_b, b_b)
    res = run_bass_kernel_spmd(nc, in_maps, list(range(NCORES)))
    global LAST_EXEC_NS
    LAST_EXEC_NS = getattr(res, "exec_time_ns", None)

    def assemble(name):
        full = np.empty((B, C, H, W), np.float32)
        for i in range(NCORES):
            blk = np.asarray(res.results[i][name], np.float32)
            blk = blk.reshape(B, RPC, C, W).transpose(0, 2, 1, 3)
            full[:, :, i * RPC:(i + 1) * RPC] = blk
        return full

    return assemble("out"), assemble("clear_out"), assemble("cf")
